# revision 24
# baseline (speedup 1.0000x reference)
"""Trainium2 Bass kernel for MixtureOfSoftmaxes (v4).

RMSNorm -> gate MLP (silu, softmax over K) -> x @ expert_w -> per-expert
softmax over vocab -> mix -> log. Vocab-sharded over 8 cores (4000
cols/expert/core, no padding); only the softmax denominators Z cross
cores (AllReduce per sweep).

v4: everything off the GEMM critical path is interleaved INTO the sweep
emission as filler thunks so each engine FIFO stays unblocked:
  sweep0 hosts the norm+transpose of blocks 2-7 and the gate down-proj;
  sweep s hosts mix/Ln/out of sweep s-1 (placed after the AllReduce has
  landed). Gate silu uses tanh (same ACT table set as exp -> no table
  swaps inside the pipeline). Row sums reduce via ACT accum-copies.
"""

import sys

sys.path.insert(0, "/opt/trn_rl_repo")

import numpy as np
import ml_dtypes

import concourse.bacc as bacc
import concourse.mybir as mybir
import concourse.tile as tile
import concourse.hw_specs as hw_specs
from concourse.bass_utils import run_bass_kernel_spmd
from concourse.masks import make_identity

AFT = mybir.ActivationFunctionType

_ORIG_GET_TABLES = hw_specs.get_activation_tables


def _patched_tables(arch):
    tabs = _ORIG_GET_TABLES(arch)
    anchor = "natural_log_exp_and_others"
    if anchor not in tabs:
        return tabs
    strip = tabs[anchor] & {AFT.Exp, AFT.Copy, AFT.Identity, AFT.Ln}
    out = {}
    for name, funcs in tabs.items():
        out[name] = funcs if name == anchor else (funcs - strip)
    return out


hw_specs.get_activation_tables = _patched_tables
bacc.get_activation_tables = _patched_tables
ALU = mybir.AluOpType
F32 = mybir.dt.float32
BF16 = mybir.dt.bfloat16
FP8 = mybir.dt.float8e4
FP8NP = ml_dtypes.float8_e4m3
WSCALE = 16.0

B, S, H, K, V = 2, 512, 1024, 4, 32000
T = B * S
NC = 8
VSH = V // NC          # 4000
C = K * VSH            # 16000
D = H // 2
EPS_RMS = 1e-5
EPS_LOG = 1e-10
TB = T // 128          # 8
HB = H // 128          # 8
HS = HB // 2           # 4
WA = 2048
WB = VSH - WA          # 1952
SWEEPS = [(0, 2), (2, 2), (4, 2), (6, 2)]  # (first block, n blocks)


def _units(wid, uw):
    out = []
    off = 0
    while off < wid:
        out.append((off, min(uw, wid - off)))
        off += uw
    return out


def build():
    nc = bacc.Bacc("TRN2", target_bir_lowering=False, debug=False, num_devices=NC)
    x_d = nc.dram_tensor("x", [T, H], F32, kind="ExternalInput")
    wa_d = nc.dram_tensor("wa", [K, HS, 128, 2, WA], FP8, kind="ExternalInput")
    wb_d = nc.dram_tensor("wb", [K, HS, 128, 2, WB], FP8, kind="ExternalInput")
    wd_d = nc.dram_tensor("wd", [HS, 128, 2, D], FP8, kind="ExternalInput")
    wu_d = nc.dram_tensor("wu", [D, K], BF16, kind="ExternalInput")
    bd_d = nc.dram_tensor("bd", [D, 1], F32, kind="ExternalInput")
    bu_d = nc.dram_tensor("bu", [128, K], F32, kind="ExternalInput")
    o_d = nc.dram_tensor("o", [TB, 128, VSH], BF16, kind="ExternalOutput")

    x_ap = x_d.rearrange("(t p) h -> t p h", p=128)
    wd_ap = wd_d.rearrange("hs p j d -> p hs j d")
    wu_ap = wu_d.rearrange("(db p) k -> p db k", p=128)
    bd_ap = bd_d.rearrange("(db p) o -> p db o", p=128)

    with tile.TileContext(nc) as tc:
        with tc.tile_pool(name="persist", bufs=1) as pers, \
             tc.tile_pool(name="pP", bufs=4) as pP, \
             tc.tile_pool(name="wmm", bufs=8) as wpool, \
             tc.tile_pool(name="mix", bufs=2) as mixp, \
             tc.tile_pool(name="ccdr", bufs=5, space="DRAM") as ccdr:
            ident = pers.tile([128, 128], BF16)
            make_identity(nc, ident[:])
            eps_rms = pers.tile([128, 1], F32)
            nc.gpsimd.memset(eps_rms[:], EPS_RMS)
            eps_log = pers.tile([128, 1], F32)
            nc.gpsimd.memset(eps_log[:], EPS_LOG)
            xT8 = pers.tile([128, HB, T], FP8)
            ss = pers.tile([128, TB], F32)
            sd = pers.tile([128, TB], F32)
            rinv = pers.tile([128, TB], F32)
            gw = pers.tile([128, TB, K], F32)
            wd_sb = pers.tile([128, HS, 2, D], FP8)
            wu_sb = pers.tile([128, D // 128, K], BF16)
            bd_sb = pers.tile([128, D // 128, 1], F32)
            nbd_sb = pers.tile([128, D // 128, 1], F32)
            bub_sb = pers.tile([128, K], F32)

            nc.sync.dma_start(wd_sb[:], wd_ap)
            nc.sync.dma_start(wu_sb[:], wu_ap)
            nc.sync.dma_start(bd_sb[:], bd_ap)
            nc.sync.dma_start(bub_sb[:], bu_d[:])

            sync0 = pers.tile([128, 1], F32)
            nc.gpsimd.memset(sync0[:], 1.0)
            bsi = ccdr.tile([128, 1], F32, tag="bsi", name="bsi")
            bso = ccdr.tile([128, 1], F32, tag="bso", name="bso")
            nc.sync.dma_start(bsi[:], sync0[:])
            nc.gpsimd.collective_compute(
                "AllReduce", ALU.add,
                replica_groups=[list(range(NC))],
                ins=[bsi[:]], outs=[bso[:]],
            )

            P_tiles = {}
            for s, (b0, nb) in enumerate(SWEEPS):
                for t2 in range(nb):
                    P_tiles[b0 + t2] = pP.tile([128, C], BF16, tag="P",
                                               name=f"P{b0 + t2}")
            bos = {}

            def norm_front(norm_pool, t):
                """x load + sum-of-squares (DVE) + bf16 cast. No ACT ops."""
                xt = norm_pool.tile([128, H], F32, tag="xt")
                nc.sync.dma_start(xt[:], x_ap[t])
                xb = norm_pool.tile([128, H], BF16, tag="xb", name=f"xb{t}")
                nc.vector.tensor_copy(xb[:], xt[:])
                nc.vector.scalar_tensor_tensor(
                    xt[:], xt[:], 0.0, xt[:], op0=ALU.bypass, op1=ALU.mult,
                    accum_out=ss[:, t : t + 1])
                return xb

            def norm_back(norm_pool, tpp, t, xb):
                """sqrt -> rinv -> diag -> 8 transpose matmuls -> fp8 copies."""
                nc.scalar.activation(sd[:, t : t + 1], ss[:, t : t + 1],
                                     AFT.Ln, bias=eps_rms[:], scale=1.0 / H)
                nc.scalar.activation(rinv[:, t : t + 1], sd[:, t : t + 1],
                                     AFT.Exp, bias=0.0, scale=-0.5)
                diag = norm_pool.tile([128, 128], BF16, tag="diag")
                nc.vector.tensor_scalar_mul(diag[:], ident[:], rinv[:, t : t + 1])
                for h in range(HB):
                    tp = tpp.tile([128, 128], F32, tag="tp")
                    nc.tensor.matmul(tp[:], lhsT=xb[:, h * 128 : (h + 1) * 128],
                                     rhs=diag[:], start=True, stop=True)
                    if h % 2 == 0:
                        nc.scalar.copy(xT8[:, h, t * 128 : (t + 1) * 128], tp[:])
                    else:
                        nc.vector.tensor_copy(
                            xT8[:, h, t * 128 : (t + 1) * 128], tp[:])

            def emit_sweep(s, mmpool, uw, pstag, fillers=None,
                           ar_split=False):
                """GEMM + exp + row sums + AllReduce; fillers run between
                supers (8 supers per sweep, index 0..7). With ar_split the
                row-sum AllReduce fires in two halves: experts 0-1 after
                super 3 (hidden under the sweep), experts 2-3 at the end."""
                b0, nb = SWEEPS[s]
                nu = (WA + WB) // uw + 1
                schunk = mixp.tile([128, nb, K * nu], F32, tag=f"sch{nb}{uw}",
                                   name=f"sch{s}")

                def emit_ar(k0, k1, tag):
                    kk = k1 - k0
                    bi = ccdr.tile([128, nb * kk * nu], F32,
                                   tag=f"bi{nb}{kk}{nu}", name=f"bi{s}{tag}")
                    bo = ccdr.tile([128, nb * kk * nu], F32,
                                   tag=f"bo{nb}{kk}{nu}", name=f"bo{s}{tag}")
                    nc.sync.dma_start(
                        bi[:].rearrange("p (t x) -> p t x", t=nb),
                        schunk[:, :, k0 * nu : k1 * nu])
                    nc.gpsimd.collective_compute(
                        "AllReduce", ALU.add,
                        replica_groups=[list(range(NC))],
                        ins=[bi[:]], outs=[bo[:]],
                    )
                    return bo

                si = 0
                for k in range(K):
                    for sup, (coff, wid, w_src) in enumerate(
                            [(0, WA, wa_d), (WA, WB, wb_d)]):
                        wts = []
                        for hs in range(HS):
                            wt = wpool.tile([128, 2, WA], FP8, tag="wt",
                                            name=f"wt{s}_{k}_{sup}_{hs}")
                            nc.sync.dma_start(wt[:, :, :wid], w_src[k, hs])
                            wts.append(wt)
                        units = _units(wid, uw)
                        if False:
                            # lhsT-shared order: all units/slices per (hs, t2)
                            pss = {}
                            for ui in range(len(units)):
                                for t2 in range(nb):
                                    pss[(ui, t2)] = mmpool.tile(
                                        [128, uw], F32, tag=f"ps{t2}",
                                        name=f"ps{s}_{k}_{sup}_{ui}_{t2}")
                            for hs in range(HS):
                                for t2 in range(nb):
                                    t = b0 + t2
                                    for ui, (uoff, ucw) in enumerate(units):
                                        for sl in range(0, ucw, 512):
                                            sw = min(512, ucw - sl)
                                            nc.tensor.matmul(
                                                pss[(ui, t2)][:, sl : sl + sw],
                                                lhsT=xT8[:, 2 * hs : 2 * hs + 2,
                                                         t * 128 : (t + 1) * 128],
                                                rhs=wts[hs][:, :, uoff + sl
                                                            : uoff + sl + sw],
                                                start=(hs == 0),
                                                stop=(hs == HS - 1),
                                                perf_mode=mybir.MatmulPerfMode.DoubleRow,
                                            )
                            for ui, (uoff, ucw) in enumerate(units):
                                acc_i = k * nu + (coff + uoff) // uw
                                for t2 in range(nb):
                                    col = k * VSH + coff + uoff
                                    nc.scalar.activation(
                                        P_tiles[b0 + t2][:, col : col + ucw],
                                        pss[(ui, t2)][:, :ucw], AFT.Exp,
                                        bias=0.0, scale=1.0 / WSCALE,
                                        accum_out=schunk[:, t2,
                                                         acc_i : acc_i + 1])
                        else:
                            for ui, (uoff, ucw) in enumerate(units):
                                pss = []
                                for t2 in range(nb):
                                    tg = pstag if pstag else f"ps{t2}"
                                    ps = mmpool.tile([128, uw], F32, tag=tg,
                                                     name=f"ps{s}_{k}_{sup}_{ui}_{t2}")
                                    pss.append(ps)
                                for hs in range(HS):
                                    for t2 in range(nb):
                                        t = b0 + t2
                                        for sl in range(0, ucw, 512):
                                            sw = min(512, ucw - sl)
                                            nc.tensor.matmul(
                                                pss[t2][:, sl : sl + sw],
                                                lhsT=xT8[:, 2 * hs : 2 * hs + 2,
                                                         t * 128 : (t + 1) * 128],
                                                rhs=wts[hs][:, :, uoff + sl
                                                            : uoff + sl + sw],
                                                start=(hs == 0),
                                                stop=(hs == HS - 1),
                                                perf_mode=mybir.MatmulPerfMode.DoubleRow,
                                            )
                                acc_i = k * nu + (coff + uoff) // uw
                                for t2 in range(nb):
                                    col = k * VSH + coff + uoff
                                    nc.scalar.activation(
                                        P_tiles[b0 + t2][:, col : col + ucw],
                                        pss[t2][:, :ucw], AFT.Exp,
                                        bias=0.0, scale=1.0 / WSCALE,
                                        accum_out=schunk[:, t2,
                                                         acc_i : acc_i + 1])
                        if ar_split and si == 3:
                            bos[(s, "a")] = emit_ar(0, 2, "a")
                        if fillers and si in fillers:
                            for th in fillers[si]:
                                th()
                        si += 1
                if ar_split:
                    bos[(s, "b")] = emit_ar(2, 4, "b")
                else:
                    bos[s] = emit_ar(0, 4, "f")

            a_tiles = {}

            def mix_pre(s):
                """z units -> reduce -> a for sweep s (DVE + one DMA)."""
                b0, nb = SWEEPS[s]
                nu = 8 if s == 0 else 4
                zr = mixp.tile([128, nb, K, nu], F32, tag=f"zr{nb}{nu}",
                               name=f"zr{s}")
                nc.sync.dma_start(zr[:].rearrange("p t k u -> p (t k u)"),
                                  bos[s][:])
                z_q = mixp.tile([128, nb, K], F32, tag=f"z{nb}", name=f"z{s}")
                nc.vector.tensor_reduce(z_q[:], zr[:],
                                        axis=mybir.AxisListType.X, op=ALU.add)
                a_q = mixp.tile([128, nb, K], F32, tag=f"a{nb}", name=f"a{s}")
                nc.vector.reciprocal(a_q[:], z_q[:])
                nc.vector.tensor_mul(a_q[:], a_q[:], gw[:, b0 : b0 + nb, :])
                a_tiles[s] = a_q

            def mix_pre_grp(s, grp, k0, k1):
                """z -> a for expert range [k0,k1) of sweep s."""
                b0, nb = SWEEPS[s]
                kk = k1 - k0
                z_q = mixp.tile([128, nb, kk], F32, tag=f"zg{kk}",
                                name=f"z{s}{grp}")
                nc.sync.dma_start(z_q[:].rearrange("p t k -> p (t k)"),
                                  bos[(s, grp)][:])
                a_q = mixp.tile([128, nb, kk], F32, tag=f"ag{kk}",
                                name=f"a{s}{grp}")
                nc.vector.reciprocal(a_q[:], z_q[:])
                nc.vector.tensor_mul(a_q[:], a_q[:],
                                     gw[:, b0 : b0 + nb, k0:k1])
                a_tiles[(s, grp)] = a_q

            def premix_blk(s, t2):
                """experts 0-1 mixed into the accumulator (needs z group a)."""
                b0, nb = SWEEPS[s]
                t = b0 + t2
                Pt = P_tiles[t]
                a_q = a_tiles[(s, "a")]
                HW2 = VSH // 2
                for hf in range(2):
                    lo = hf * HW2
                    for k in range(2):
                        nc.vector.tensor_scalar_mul(
                            Pt[:, k * VSH + lo : k * VSH + lo + HW2],
                            Pt[:, k * VSH + lo : k * VSH + lo + HW2],
                            a_q[:, t2, k : k + 1])
                    nc.vector.tensor_tensor(
                        Pt[:, lo : lo + HW2], Pt[:, lo : lo + HW2],
                        Pt[:, VSH + lo : VSH + lo + HW2], op=ALU.add)

            def postmix_blk(s, t2):
                """experts 2-3 folded in + Ln + out (needs z group b)."""
                b0, nb = SWEEPS[s]
                t = b0 + t2
                Pt = P_tiles[t]
                a_q = a_tiles[(s, "b")]
                HW2 = VSH // 2
                for hf in range(2):
                    lo = hf * HW2
                    for k in range(2, K):
                        nc.vector.tensor_scalar_mul(
                            Pt[:, k * VSH + lo : k * VSH + lo + HW2],
                            Pt[:, k * VSH + lo : k * VSH + lo + HW2],
                            a_q[:, t2, k - 2 : k - 1])
                    for k in range(2, K):
                        nc.vector.tensor_tensor(
                            Pt[:, lo : lo + HW2], Pt[:, lo : lo + HW2],
                            Pt[:, k * VSH + lo : k * VSH + lo + HW2],
                            op=ALU.add)
                    ot = mixp.tile([128, HW2], BF16, tag="ot",
                                   name=f"ot{t}_{hf}")
                    nc.scalar.activation(ot[:], Pt[:, lo : lo + HW2],
                                         AFT.Ln, bias=eps_log[:], scale=1.0)
                    nc.sync.dma_start(o_d[t, :, lo : lo + HW2], ot[:])

            def mix_blk(s, t2):
                """in-place mix -> Ln -> out DMA, pipelined per 2000-col half."""
                b0, nb = SWEEPS[s]
                t = b0 + t2
                Pt = P_tiles[t]
                a_q = a_tiles[s]
                HW2 = VSH // 2
                for hf in range(2):
                    lo = hf * HW2
                    for k in range(K):
                        nc.vector.tensor_scalar_mul(
                            Pt[:, k * VSH + lo : k * VSH + lo + HW2],
                            Pt[:, k * VSH + lo : k * VSH + lo + HW2],
                            a_q[:, t2, k : k + 1])
                    for k in range(1, K):
                        nc.vector.tensor_tensor(
                            Pt[:, lo : lo + HW2], Pt[:, lo : lo + HW2],
                            Pt[:, k * VSH + lo : k * VSH + lo + HW2],
                            op=ALU.add)
                    ot = mixp.tile([128, HW2], BF16, tag="ot",
                                   name=f"ot{t}_{hf}")
                    nc.scalar.activation(ot[:], Pt[:, lo : lo + HW2],
                                         AFT.Ln, bias=eps_log[:], scale=1.0)
                    nc.sync.dma_start(o_d[t, :, lo : lo + HW2], ot[:])

            # ---------------- region 1: preamble + sweep0 + gate ----------
            with tc.tile_pool(name="norm", bufs=2) as norm_pool, \
                 tc.tile_pool(name="tp_psum", bufs=2, space="PSUM") as tpp, \
                 tc.tile_pool(name="mm0_psum", bufs=3, space="PSUM") as mm0, \
                 tc.tile_pool(name="gate_psum", bufs=2, space="PSUM") as gps, \
                 tc.tile_pool(name="gate_sb", bufs=1) as gsb:
                xbs = {}
                for t in (0, 1):
                    xbs[t] = norm_front(norm_pool, t)
                    norm_back(norm_pool, tpp, t, xbs[t])
                for t in range(2, TB):
                    xbs[t] = norm_front(norm_pool, t)
                # silu = x * (tanh(x/2) + 1) / 2, tanh shares exp's table set
                nc.vector.tensor_scalar_mul(nbd_sb[:], bd_sb[:], -1.0)
                gT = gsb.tile([128, D // 128, T], BF16)

                def gate_down(d):
                    def th():
                        for half in range(2):
                            pg = gps.tile([128, 512], F32, tag="pg",
                                          name=f"pg{d}_{half}", bufs=2)
                            for hs in range(HS):
                                nc.tensor.matmul(
                                    pg[:],
                                    lhsT=wd_sb[:, hs, :, d * 128 : (d + 1) * 128],
                                    rhs=xT8[:, 2 * hs : 2 * hs + 2,
                                            half * 512 : (half + 1) * 512],
                                    start=(hs == 0), stop=(hs == HS - 1),
                                    perf_mode=mybir.MatmulPerfMode.DoubleRow,
                                )
                            eg = gsb.tile([128, 512], F32, tag="eg",
                                          name=f"eg{d}_{half}", bufs=1)
                            nc.scalar.activation(eg[:], pg[:], AFT.Exp,
                                                 bias=nbd_sb[:, d, :],
                                                 scale=-1.0 / WSCALE)
                            lin = gsb.tile([128, 512], BF16, tag="lin",
                                           name=f"lin{d}_{half}", bufs=1)
                            with nc.allow_low_precision(
                                    reason="gate lin in bf16 is plenty"):
                                nc.vector.tensor_scalar(
                                    lin[:], pg[:], 1.0 / WSCALE,
                                    bd_sb[:, d, :],
                                    op0=ALU.mult, op1=ALU.add)
                            nc.vector.tensor_scalar_add(eg[:], eg[:], 1.0)
                            nc.vector.reciprocal_approx_fast(eg[:], eg[:])
                            nc.vector.tensor_mul(
                                gT[:, d, half * 512 : (half + 1) * 512],
                                lin[:], eg[:])
                    return th

                glt = gsb.tile([128, TB, K], F32)

                def gate_up(ts):
                    def th():
                        for t in ts:
                            pl = gps.tile([128, K], F32, tag="pl",
                                          name=f"pl{t}", bufs=1)
                            for d in range(D // 128):
                                nc.tensor.matmul(
                                    pl[:],
                                    lhsT=gT[:, d, t * 128 : (t + 1) * 128],
                                    rhs=wu_sb[:, d, :],
                                    start=(d == 0), stop=(d == D // 128 - 1),
                                )
                            nc.vector.scalar_tensor_tensor(
                                glt[:, t, :], pl[:], 1.0, bub_sb[:],
                                op0=ALU.bypass, op1=ALU.add)
                    return th

                def gate_softmax():
                    negm = gsb.tile([128, TB], F32)
                    esum = gsb.tile([128, TB], F32)
                    for t in range(TB):
                        nc.vector.tensor_reduce(
                            negm[:, t : t + 1], glt[:, t, :],
                            axis=mybir.AxisListType.X, op=ALU.max, negate=True)
                        nc.scalar.activation(gw[:, t, :], glt[:, t, :], AFT.Exp,
                                             bias=negm[:, t : t + 1], scale=1.0,
                                             accum_out=esum[:, t : t + 1])
                    rsum = gsb.tile([128, TB], F32)
                    nc.vector.reciprocal(rsum[:], esum[:])
                    for t in range(TB):
                        nc.vector.tensor_scalar_mul(gw[:, t, :], gw[:, t, :],
                                                    rsum[:, t : t + 1])

                def nb_thunk(t):
                    return lambda: norm_back(norm_pool, tpp, t, xbs[t])

                fill0 = {
                    0: [nb_thunk(2), nb_thunk(3)],
                    1: [nb_thunk(4), nb_thunk(5)],
                    2: [nb_thunk(6), nb_thunk(7)],
                    3: [gate_down(0), gate_down(1)],
                    4: [gate_down(2), gate_down(3)],
                    5: [gate_up(range(0, 4))],
                    6: [gate_up(range(4, TB))],
                    7: [gate_softmax],
                }
                emit_sweep(0, mm0, 512, "ps", fillers=fill0)

            # ---------------- region 2: sweeps 1..4, interleaved mixes ----
            with tc.tile_pool(name="mm_psum", bufs=2, space="PSUM") as mmps:
                emit_sweep(1, mmps, 1024, None, fillers={
                    6: [lambda: mix_pre(0), lambda: mix_blk(0, 0)],
                    7: [lambda: mix_blk(0, 1)]})
                emit_sweep(2, mmps, 1024, None, fillers={
                    6: [lambda: mix_pre(1), lambda: mix_blk(1, 0)],
                    7: [lambda: mix_blk(1, 1)]})
                emit_sweep(3, mmps, 1024, None, fillers={
                    6: [lambda: mix_pre(2), lambda: mix_blk(2, 0)],
                    7: [lambda: mix_blk(2, 1)]})
                mix_pre(3)
                # tail mix: block 6 on DVE, block 7's adds on GPSIMD (idle),
                # muls interleaved so both engines start their adds early
                a_q = a_tiles[3]
                P6, P7 = P_tiles[6], P_tiles[7]
                HW2 = VSH // 2
                for hf in range(2):
                    lo = hf * HW2
                    for Pt, t2 in ((P7, 1), (P6, 0)):
                        for k in range(K):
                            nc.vector.tensor_scalar_mul(
                                Pt[:, k * VSH + lo : k * VSH + lo + HW2],
                                Pt[:, k * VSH + lo : k * VSH + lo + HW2],
                                a_q[:, t2, k : k + 1])
                for hf in range(2):
                    lo = hf * HW2
                    for k in range(1, K):
                        nc.gpsimd.tensor_tensor(
                            P7[:, lo : lo + HW2], P7[:, lo : lo + HW2],
                            P7[:, k * VSH + lo : k * VSH + lo + HW2],
                            op=ALU.add)
                    ot7 = mixp.tile([128, HW2], BF16, tag="ot",
                                    name=f"ot7_{hf}")
                    nc.scalar.activation(ot7[:], P7[:, lo : lo + HW2],
                                         AFT.Ln, bias=eps_log[:], scale=1.0)
                    nc.sync.dma_start(o_d[7, :, lo : lo + HW2], ot7[:])
                    for k in range(1, K):
                        nc.vector.tensor_tensor(
                            P6[:, lo : lo + HW2], P6[:, lo : lo + HW2],
                            P6[:, k * VSH + lo : k * VSH + lo + HW2],
                            op=ALU.add)
                    ot6 = mixp.tile([128, HW2], BF16, tag="ot",
                                    name=f"ot6_{hf}")
                    nc.scalar.activation(ot6[:], P6[:, lo : lo + HW2],
                                         AFT.Ln, bias=eps_log[:], scale=1.0)
                    nc.sync.dma_start(o_d[6, :, lo : lo + HW2], ot6[:])
    nc.compile()
    return nc


_CACHE = {}


def _get_kernel():
    if "k" not in _CACHE:
        _CACHE["k"] = build()
    return _CACHE["k"]


def kernel(hidden_states, rms_scale, gate_down_w, gate_down_b, gate_up_w,
           gate_up_b, expert_w, trace=False):
    nc_k = _get_kernel()
    core_ids = list(range(NC))

    x = np.ascontiguousarray(
        np.asarray(hidden_states, dtype=np.float32).reshape(T, H))
    scale = np.asarray(rms_scale, dtype=np.float32)
    wd_f = np.asarray(gate_down_w, dtype=np.float32) * scale[:, None]
    wd8 = np.ascontiguousarray(
        (wd_f * WSCALE).reshape(HS, 2, 128, D).transpose(0, 2, 1, 3)
    ).astype(FP8NP)
    wu = np.asarray(gate_up_w, dtype=np.float32).astype(ml_dtypes.bfloat16)
    bd = np.ascontiguousarray(
        np.asarray(gate_down_b, dtype=np.float32).reshape(D, 1))
    bu = np.ascontiguousarray(np.broadcast_to(
        np.asarray(gate_up_b, dtype=np.float32).reshape(1, K), (128, K)).copy())
    we = np.asarray(expert_w, dtype=np.float32) * (scale[:, None] * WSCALE)

    in_maps = []
    for c in range(NC):
        wa = np.empty((K, HS, 128, 2, WA), dtype=FP8NP)
        wb = np.empty((K, HS, 128, 2, WB), dtype=FP8NP)
        for k in range(K):
            blk = we[:, k * V + c * VSH : k * V + (c + 1) * VSH]
            fr = blk.reshape(HS, 2, 128, VSH).transpose(0, 2, 1, 3)
            wa[k] = fr[:, :, :, :WA].astype(FP8NP)
            wb[k] = fr[:, :, :, WA:].astype(FP8NP)
        in_maps.append({"x": x, "wa": wa, "wb": wb, "wd": wd8, "wu": wu,
                        "bd": bd, "bu": bu})

    res = run_bass_kernel_spmd(nc_k, in_maps, core_ids, trace=trace)

    out = np.empty((T, V), dtype=np.float32)
    for c in range(NC):
        out[:, c * VSH : (c + 1) * VSH] = (
            res.results[c]["o"].astype(np.float32).reshape(T, VSH))
    out = out.reshape(B, S, V)
    if trace:
        return out, (res, res)
    return out


# revision 27
# speedup vs baseline: 1.0114x; 1.0114x over previous
"""Trainium2 Bass kernel for MixtureOfSoftmaxes (v4).

RMSNorm -> gate MLP (silu, softmax over K) -> x @ expert_w -> per-expert
softmax over vocab -> mix -> log. Vocab-sharded over 8 cores (4000
cols/expert/core, no padding); only the softmax denominators Z cross
cores (AllReduce per sweep).

v4: everything off the GEMM critical path is interleaved INTO the sweep
emission as filler thunks so each engine FIFO stays unblocked:
  sweep0 hosts the norm+transpose of blocks 2-7 and the gate down-proj;
  sweep s hosts mix/Ln/out of sweep s-1 (placed after the AllReduce has
  landed). Gate silu uses tanh (same ACT table set as exp -> no table
  swaps inside the pipeline). Row sums reduce via ACT accum-copies.
"""

import sys

sys.path.insert(0, "/opt/trn_rl_repo")

import numpy as np
import ml_dtypes

import concourse.bacc as bacc
import concourse.mybir as mybir
import concourse.tile as tile
import concourse.hw_specs as hw_specs
from concourse.bass_utils import run_bass_kernel_spmd
from concourse.masks import make_identity

AFT = mybir.ActivationFunctionType

_ORIG_GET_TABLES = hw_specs.get_activation_tables


def _patched_tables(arch):
    tabs = _ORIG_GET_TABLES(arch)
    anchor = "natural_log_exp_and_others"
    if anchor not in tabs:
        return tabs
    strip = tabs[anchor] & {AFT.Exp, AFT.Copy, AFT.Identity, AFT.Ln}
    out = {}
    for name, funcs in tabs.items():
        out[name] = funcs if name == anchor else (funcs - strip)
    return out


hw_specs.get_activation_tables = _patched_tables
bacc.get_activation_tables = _patched_tables
ALU = mybir.AluOpType
F32 = mybir.dt.float32
BF16 = mybir.dt.bfloat16
FP8 = mybir.dt.float8e4
FP8NP = ml_dtypes.float8_e4m3
WSCALE = 16.0

B, S, H, K, V = 2, 512, 1024, 4, 32000
T = B * S
NC = 8
VSH = V // NC          # 4000
C = K * VSH            # 16000
D = H // 2
EPS_RMS = 1e-5
EPS_LOG = 1e-10
TB = T // 128          # 8
HB = H // 128          # 8
HS = HB // 2           # 4
WA = 2048
WB = VSH - WA          # 1952
SWEEPS = [(0, 2), (2, 2), (4, 2), (6, 2)]  # (first block, n blocks)


def _units(wid, uw):
    out = []
    off = 0
    while off < wid:
        out.append((off, min(uw, wid - off)))
        off += uw
    return out


def build():
    nc = bacc.Bacc("TRN2", target_bir_lowering=False, debug=False, num_devices=NC)
    x_d = nc.dram_tensor("x", [T, H], F32, kind="ExternalInput")
    wa_d = nc.dram_tensor("wa", [K, HS, 128, 2, WA], FP8, kind="ExternalInput")
    wb_d = nc.dram_tensor("wb", [K, HS, 128, 2, WB], FP8, kind="ExternalInput")
    wd_d = nc.dram_tensor("wd", [HS, 128, 2, D], FP8, kind="ExternalInput")
    wu_d = nc.dram_tensor("wu", [D, K], BF16, kind="ExternalInput")
    bd_d = nc.dram_tensor("bd", [D, 1], F32, kind="ExternalInput")
    bu_d = nc.dram_tensor("bu", [128, K], F32, kind="ExternalInput")
    o_d = nc.dram_tensor("o", [TB, 128, VSH], BF16, kind="ExternalOutput")

    x_ap = x_d.rearrange("(t p) h -> t p h", p=128)
    wd_ap = wd_d.rearrange("hs p j d -> p hs j d")
    wu_ap = wu_d.rearrange("(db p) k -> p db k", p=128)
    bd_ap = bd_d.rearrange("(db p) o -> p db o", p=128)

    with tile.TileContext(nc) as tc:
        with tc.tile_pool(name="persist", bufs=1) as pers, \
             tc.tile_pool(name="pP", bufs=4) as pP, \
             tc.tile_pool(name="wmm", bufs=8) as wpool, \
             tc.tile_pool(name="mix", bufs=2) as mixp, \
             tc.tile_pool(name="ccdr", bufs=5, space="DRAM") as ccdr:
            ident = pers.tile([128, 128], BF16)
            make_identity(nc, ident[:])
            eps_rms = pers.tile([128, 1], F32)
            nc.gpsimd.memset(eps_rms[:], EPS_RMS)
            eps_log = pers.tile([128, 1], F32)
            nc.gpsimd.memset(eps_log[:], EPS_LOG)
            dsc = pers.tile([128, 1], F32)
            nc.scalar.activation(dsc[:], eps_rms[:], AFT.Exp)
            xT8 = pers.tile([128, HB, T], FP8)
            ss = pers.tile([128, TB], F32)
            sd = pers.tile([128, TB], F32)
            rinv = pers.tile([128, TB], F32)
            gw = pers.tile([128, TB, K], F32)
            wd_sb = pers.tile([128, HS, 2, D], FP8)
            wu_sb = pers.tile([128, D // 128, K], BF16)
            bd_sb = pers.tile([128, D // 128, 1], F32)
            nbd_sb = pers.tile([128, D // 128, 1], F32)
            bub_sb = pers.tile([128, K], F32)

            sync0 = pers.tile([128, 1], F32)
            nc.gpsimd.memset(sync0[:], 1.0)
            bsi = ccdr.tile([128, 1], F32, tag="bsi", name="bsi")
            bso = ccdr.tile([128, 1], F32, tag="bso", name="bso")
            nc.sync.dma_start(bsi[:], sync0[:])
            nc.gpsimd.collective_compute(
                "AllReduce", ALU.add,
                replica_groups=[list(range(NC))],
                ins=[bsi[:]], outs=[bso[:]],
            )

            P_tiles = {}
            for s, (b0, nb) in enumerate(SWEEPS):
                for t2 in range(nb):
                    P_tiles[b0 + t2] = pP.tile([128, C], BF16, tag="P",
                                               name=f"P{b0 + t2}")
            bos = {}

            def norm_front(norm_pool, t):
                """x load + sum-of-squares (DVE) + bf16 cast. No ACT ops."""
                xt = norm_pool.tile([128, H], F32, tag="xt")
                nc.sync.dma_start(xt[:], x_ap[t])
                xb = norm_pool.tile([128, H], BF16, tag="xb", name=f"xb{t}")
                nc.vector.tensor_copy(xb[:], xt[:])
                nc.vector.scalar_tensor_tensor(
                    xt[:], xt[:], 0.0, xt[:], op0=ALU.bypass, op1=ALU.mult,
                    accum_out=ss[:, t : t + 1])
                return xb

            def norm_back(norm_pool, tpp, t, xb):
                """sqrt -> rinv -> diag -> 8 transpose matmuls -> fp8 copies."""
                nc.scalar.activation(sd[:, t : t + 1], ss[:, t : t + 1],
                                     AFT.Ln, bias=eps_rms[:], scale=1.0 / H)
                nc.scalar.activation(rinv[:, t : t + 1], sd[:, t : t + 1],
                                     AFT.Exp, bias=0.0, scale=-0.5)
                diag = norm_pool.tile([128, 128], BF16, tag="diag")
                nc.vector.tensor_scalar_mul(diag[:], ident[:], rinv[:, t : t + 1])
                for h in range(HB):
                    tp = tpp.tile([128, 128], F32, tag="tp")
                    nc.tensor.matmul(tp[:], lhsT=xb[:, h * 128 : (h + 1) * 128],
                                     rhs=diag[:], start=True, stop=True)
                    if h % 2 == 0:
                        nc.scalar.copy(xT8[:, h, t * 128 : (t + 1) * 128], tp[:])
                    else:
                        nc.vector.tensor_copy(
                            xT8[:, h, t * 128 : (t + 1) * 128], tp[:])

            def emit_sweep(s, mmpool, uw, pstag, fillers=None,
                           ar_split=False):
                """GEMM + exp + row sums + AllReduce; fillers run between
                supers (8 supers per sweep, index 0..7). With ar_split the
                row-sum AllReduce fires in two halves: experts 0-1 after
                super 3 (hidden under the sweep), experts 2-3 at the end."""
                b0, nb = SWEEPS[s]
                nu = (WA + WB) // uw + 1
                schunk = mixp.tile([128, nb, K * nu], F32, tag=f"sch{nb}{uw}",
                                   name=f"sch{s}")

                def emit_ar(k0, k1, tag):
                    kk = k1 - k0
                    bi = ccdr.tile([128, nb * kk * nu], F32,
                                   tag=f"bi{nb}{kk}{nu}", name=f"bi{s}{tag}")
                    bo = ccdr.tile([128, nb * kk * nu], F32,
                                   tag=f"bo{nb}{kk}{nu}", name=f"bo{s}{tag}")
                    nc.sync.dma_start(
                        bi[:].rearrange("p (t x) -> p t x", t=nb),
                        schunk[:, :, k0 * nu : k1 * nu])
                    nc.gpsimd.collective_compute(
                        "AllReduce", ALU.add,
                        replica_groups=[list(range(NC))],
                        ins=[bi[:]], outs=[bo[:]],
                    )
                    return bo

                si = 0
                for k in range(K):
                    for sup, (coff, wid, w_src) in enumerate(
                            [(0, WA, wa_d), (WA, WB, wb_d)]):
                        wts = []
                        for hs in range(HS):
                            wt = wpool.tile([128, 2, WA], FP8, tag="wt",
                                            name=f"wt{s}_{k}_{sup}_{hs}")
                            nc.sync.dma_start(wt[:, :, :wid], w_src[k, hs])
                            wts.append(wt)
                        units = _units(wid, uw)
                        if False:
                            # lhsT-shared order: all units/slices per (hs, t2)
                            pss = {}
                            for ui in range(len(units)):
                                for t2 in range(nb):
                                    pss[(ui, t2)] = mmpool.tile(
                                        [128, uw], F32, tag=f"ps{t2}",
                                        name=f"ps{s}_{k}_{sup}_{ui}_{t2}")
                            for hs in range(HS):
                                for t2 in range(nb):
                                    t = b0 + t2
                                    for ui, (uoff, ucw) in enumerate(units):
                                        for sl in range(0, ucw, 512):
                                            sw = min(512, ucw - sl)
                                            nc.tensor.matmul(
                                                pss[(ui, t2)][:, sl : sl + sw],
                                                lhsT=xT8[:, 2 * hs : 2 * hs + 2,
                                                         t * 128 : (t + 1) * 128],
                                                rhs=wts[hs][:, :, uoff + sl
                                                            : uoff + sl + sw],
                                                start=(hs == 0),
                                                stop=(hs == HS - 1),
                                                perf_mode=mybir.MatmulPerfMode.DoubleRow,
                                            )
                            for ui, (uoff, ucw) in enumerate(units):
                                acc_i = k * nu + (coff + uoff) // uw
                                for t2 in range(nb):
                                    col = k * VSH + coff + uoff
                                    nc.scalar.activation(
                                        P_tiles[b0 + t2][:, col : col + ucw],
                                        pss[(ui, t2)][:, :ucw], AFT.Exp,
                                        bias=0.0, scale=1.0 / WSCALE,
                                        accum_out=schunk[:, t2,
                                                         acc_i : acc_i + 1])
                        else:
                            for ui, (uoff, ucw) in enumerate(units):
                                pss = []
                                for t2 in range(nb):
                                    tg = pstag if pstag else f"ps{t2}"
                                    ps = mmpool.tile([128, uw], F32, tag=tg,
                                                     name=f"ps{s}_{k}_{sup}_{ui}_{t2}")
                                    pss.append(ps)
                                for hs in range(HS):
                                    for t2 in range(nb):
                                        t = b0 + t2
                                        for sl in range(0, ucw, 512):
                                            sw = min(512, ucw - sl)
                                            nc.tensor.matmul(
                                                pss[t2][:, sl : sl + sw],
                                                lhsT=xT8[:, 2 * hs : 2 * hs + 2,
                                                         t * 128 : (t + 1) * 128],
                                                rhs=wts[hs][:, :, uoff + sl
                                                            : uoff + sl + sw],
                                                start=(hs == 0),
                                                stop=(hs == HS - 1),
                                                perf_mode=mybir.MatmulPerfMode.DoubleRow,
                                            )
                                acc_i = k * nu + (coff + uoff) // uw
                                for t2 in range(nb):
                                    col = k * VSH + coff + uoff
                                    nc.scalar.activation(
                                        P_tiles[b0 + t2][:, col : col + ucw],
                                        pss[t2][:, :ucw], AFT.Exp,
                                        bias=0.0, scale=1.0 / WSCALE,
                                        accum_out=schunk[:, t2,
                                                         acc_i : acc_i + 1])
                        if ar_split and si == 3:
                            bos[(s, "a")] = emit_ar(0, 2, "a")
                        if fillers and si in fillers:
                            for th in fillers[si]:
                                th()
                        si += 1
                if ar_split:
                    bos[(s, "b")] = emit_ar(2, 4, "b")
                else:
                    bos[s] = emit_ar(0, 4, "f")

            a_tiles = {}

            def mix_pre(s):
                """z units -> reduce -> a for sweep s (DVE + one DMA)."""
                b0, nb = SWEEPS[s]
                nu = 8 if s == 0 else 4
                zr = mixp.tile([128, nb, K, nu], F32, tag=f"zr{nb}{nu}",
                               name=f"zr{s}")
                nc.sync.dma_start(zr[:].rearrange("p t k u -> p (t k u)"),
                                  bos[s][:])
                z_q = mixp.tile([128, nb, K], F32, tag=f"z{nb}", name=f"z{s}")
                nc.vector.tensor_reduce(z_q[:], zr[:],
                                        axis=mybir.AxisListType.X, op=ALU.add)
                a_q = mixp.tile([128, nb, K], F32, tag=f"a{nb}", name=f"a{s}")
                nc.vector.reciprocal(a_q[:], z_q[:])
                nc.vector.tensor_mul(a_q[:], a_q[:], gw[:, b0 : b0 + nb, :])
                a_tiles[s] = a_q

            def mix_pre_grp(s, grp, k0, k1):
                """z -> a for expert range [k0,k1) of sweep s."""
                b0, nb = SWEEPS[s]
                kk = k1 - k0
                z_q = mixp.tile([128, nb, kk], F32, tag=f"zg{kk}",
                                name=f"z{s}{grp}")
                nc.sync.dma_start(z_q[:].rearrange("p t k -> p (t k)"),
                                  bos[(s, grp)][:])
                a_q = mixp.tile([128, nb, kk], F32, tag=f"ag{kk}",
                                name=f"a{s}{grp}")
                nc.vector.reciprocal(a_q[:], z_q[:])
                nc.vector.tensor_mul(a_q[:], a_q[:],
                                     gw[:, b0 : b0 + nb, k0:k1])
                a_tiles[(s, grp)] = a_q

            def premix_blk(s, t2):
                """experts 0-1 mixed into the accumulator (needs z group a)."""
                b0, nb = SWEEPS[s]
                t = b0 + t2
                Pt = P_tiles[t]
                a_q = a_tiles[(s, "a")]
                HW2 = VSH // 2
                for hf in range(2):
                    lo = hf * HW2
                    for k in range(2):
                        nc.vector.tensor_scalar_mul(
                            Pt[:, k * VSH + lo : k * VSH + lo + HW2],
                            Pt[:, k * VSH + lo : k * VSH + lo + HW2],
                            a_q[:, t2, k : k + 1])
                    nc.vector.tensor_tensor(
                        Pt[:, lo : lo + HW2], Pt[:, lo : lo + HW2],
                        Pt[:, VSH + lo : VSH + lo + HW2], op=ALU.add)

            def postmix_blk(s, t2):
                """experts 2-3 folded in + Ln + out (needs z group b)."""
                b0, nb = SWEEPS[s]
                t = b0 + t2
                Pt = P_tiles[t]
                a_q = a_tiles[(s, "b")]
                HW2 = VSH // 2
                for hf in range(2):
                    lo = hf * HW2
                    for k in range(2, K):
                        nc.vector.tensor_scalar_mul(
                            Pt[:, k * VSH + lo : k * VSH + lo + HW2],
                            Pt[:, k * VSH + lo : k * VSH + lo + HW2],
                            a_q[:, t2, k - 2 : k - 1])
                    for k in range(2, K):
                        nc.vector.tensor_tensor(
                            Pt[:, lo : lo + HW2], Pt[:, lo : lo + HW2],
                            Pt[:, k * VSH + lo : k * VSH + lo + HW2],
                            op=ALU.add)
                    ot = mixp.tile([128, HW2], BF16, tag="ot",
                                   name=f"ot{t}_{hf}")
                    nc.scalar.activation(ot[:], Pt[:, lo : lo + HW2],
                                         AFT.Ln, bias=eps_log[:], scale=1.0)
                    nc.sync.dma_start(o_d[t, :, lo : lo + HW2], ot[:])

            def mix_blk(s, t2):
                """in-place mix -> Ln -> out DMA, pipelined per 2000-col half."""
                b0, nb = SWEEPS[s]
                t = b0 + t2
                Pt = P_tiles[t]
                a_q = a_tiles[s]
                HW2 = VSH // 2
                for hf in range(2):
                    lo = hf * HW2
                    for k in range(K):
                        nc.vector.tensor_scalar_mul(
                            Pt[:, k * VSH + lo : k * VSH + lo + HW2],
                            Pt[:, k * VSH + lo : k * VSH + lo + HW2],
                            a_q[:, t2, k : k + 1])
                    for k in range(1, K):
                        nc.vector.tensor_tensor(
                            Pt[:, lo : lo + HW2], Pt[:, lo : lo + HW2],
                            Pt[:, k * VSH + lo : k * VSH + lo + HW2],
                            op=ALU.add)
                    ot = mixp.tile([128, HW2], BF16, tag="ot",
                                   name=f"ot{t}_{hf}")
                    nc.scalar.activation(ot[:], Pt[:, lo : lo + HW2],
                                         AFT.Ln, bias=eps_log[:], scale=1.0)
                    nc.sync.dma_start(o_d[t, :, lo : lo + HW2], ot[:])

            # ---------------- region 1: preamble + sweep0 + gate ----------
            with tc.tile_pool(name="norm", bufs=2) as norm_pool, \
                 tc.tile_pool(name="tp_psum", bufs=2, space="PSUM") as tpp, \
                 tc.tile_pool(name="mm0_psum", bufs=3, space="PSUM") as mm0, \
                 tc.tile_pool(name="gate_psum", bufs=2, space="PSUM") as gps, \
                 tc.tile_pool(name="gate_sb", bufs=1) as gsb:
                xbs = {}
                for t in (0, 1):
                    xbs[t] = norm_front(norm_pool, t)
                nc.sync.dma_start(wd_sb[:], wd_ap)
                nc.sync.dma_start(wu_sb[:], wu_ap)
                nc.sync.dma_start(bd_sb[:], bd_ap)
                nc.sync.dma_start(bub_sb[:], bu_d[:])
                for t in (0, 1):
                    norm_back(norm_pool, tpp, t, xbs[t])
                for t in range(2, TB):
                    xbs[t] = norm_front(norm_pool, t)
                # silu = x * (tanh(x/2) + 1) / 2, tanh shares exp's table set
                nc.vector.tensor_scalar_mul(nbd_sb[:], bd_sb[:], -1.0)
                gT = gsb.tile([128, D // 128, T], BF16)

                def gate_down(d):
                    def th():
                        for half in range(2):
                            pg = gps.tile([128, 512], F32, tag="pg",
                                          name=f"pg{d}_{half}", bufs=2)
                            for hs in range(HS):
                                nc.tensor.matmul(
                                    pg[:],
                                    lhsT=wd_sb[:, hs, :, d * 128 : (d + 1) * 128],
                                    rhs=xT8[:, 2 * hs : 2 * hs + 2,
                                            half * 512 : (half + 1) * 512],
                                    start=(hs == 0), stop=(hs == HS - 1),
                                    perf_mode=mybir.MatmulPerfMode.DoubleRow,
                                )
                            eg = gsb.tile([128, 512], F32, tag="eg",
                                          name=f"eg{d}_{half}", bufs=1)
                            nc.scalar.activation(eg[:], pg[:], AFT.Exp,
                                                 bias=nbd_sb[:, d, :],
                                                 scale=-1.0 / WSCALE)
                            lin = gsb.tile([128, 512], BF16, tag="lin",
                                           name=f"lin{d}_{half}", bufs=1)
                            with nc.allow_low_precision(
                                    reason="gate lin in bf16 is plenty"):
                                nc.vector.tensor_scalar(
                                    lin[:], pg[:], 1.0 / WSCALE,
                                    bd_sb[:, d, :],
                                    op0=ALU.mult, op1=ALU.add)
                            nc.vector.tensor_scalar_add(eg[:], eg[:], 1.0)
                            nc.vector.reciprocal_approx_fast(eg[:], eg[:])
                            nc.vector.tensor_mul(
                                gT[:, d, half * 512 : (half + 1) * 512],
                                lin[:], eg[:])
                    return th

                glt = gsb.tile([128, TB, K], F32)

                def gate_up(ts):
                    def th():
                        for t in ts:
                            pl = gps.tile([128, K], F32, tag="pl",
                                          name=f"pl{t}", bufs=1)
                            for d in range(D // 128):
                                nc.tensor.matmul(
                                    pl[:],
                                    lhsT=gT[:, d, t * 128 : (t + 1) * 128],
                                    rhs=wu_sb[:, d, :],
                                    start=(d == 0), stop=(d == D // 128 - 1),
                                )
                            nc.vector.scalar_tensor_tensor(
                                glt[:, t, :], pl[:], 1.0, bub_sb[:],
                                op0=ALU.bypass, op1=ALU.add)
                    return th

                def gate_softmax():
                    negm = gsb.tile([128, TB], F32)
                    esum = gsb.tile([128, TB], F32)
                    for t in range(TB):
                        nc.vector.tensor_reduce(
                            negm[:, t : t + 1], glt[:, t, :],
                            axis=mybir.AxisListType.X, op=ALU.max, negate=True)
                        nc.scalar.activation(gw[:, t, :], glt[:, t, :], AFT.Exp,
                                             bias=negm[:, t : t + 1], scale=1.0,
                                             accum_out=esum[:, t : t + 1])
                    rsum = gsb.tile([128, TB], F32)
                    nc.vector.reciprocal(rsum[:], esum[:])
                    for t in range(TB):
                        nc.vector.tensor_scalar_mul(gw[:, t, :], gw[:, t, :],
                                                    rsum[:, t : t + 1])

                def nb_thunk(t):
                    return lambda: norm_back(norm_pool, tpp, t, xbs[t])

                fill0 = {
                    0: [nb_thunk(2), nb_thunk(3)],
                    1: [nb_thunk(4), nb_thunk(5)],
                    2: [nb_thunk(6), nb_thunk(7)],
                    3: [gate_down(0), gate_down(1)],
                    4: [gate_down(2), gate_down(3)],
                    5: [gate_up(range(0, 4))],
                    6: [gate_up(range(4, TB))],
                    7: [gate_softmax],
                }
                emit_sweep(0, mm0, 512, "ps", fillers=fill0)

            # ---------------- region 2: sweeps 1..4, interleaved mixes ----
            with tc.tile_pool(name="mm_psum", bufs=2, space="PSUM") as mmps:
                emit_sweep(1, mmps, 1024, None, fillers={
                    6: [lambda: mix_pre(0), lambda: mix_blk(0, 0)],
                    7: [lambda: mix_blk(0, 1)]})
                emit_sweep(2, mmps, 1024, None, fillers={
                    6: [lambda: mix_pre(1), lambda: mix_blk(1, 0)],
                    7: [lambda: mix_blk(1, 1)]})
                emit_sweep(3, mmps, 1024, None, fillers={
                    6: [lambda: mix_pre(2), lambda: mix_blk(2, 0)],
                    7: [lambda: mix_blk(2, 1)]})
                mix_pre(3)
                mix_blk(3, 0)
                mix_blk(3, 1)
    nc.compile()
    return nc


_CACHE = {}


def _get_kernel():
    if "k" not in _CACHE:
        _CACHE["k"] = build()
    return _CACHE["k"]


def kernel(hidden_states, rms_scale, gate_down_w, gate_down_b, gate_up_w,
           gate_up_b, expert_w, trace=False):
    nc_k = _get_kernel()
    core_ids = list(range(NC))

    x = np.ascontiguousarray(
        np.asarray(hidden_states, dtype=np.float32).reshape(T, H))
    scale = np.asarray(rms_scale, dtype=np.float32)
    wd_f = np.asarray(gate_down_w, dtype=np.float32) * scale[:, None]
    wd8 = np.ascontiguousarray(
        (wd_f * WSCALE).reshape(HS, 2, 128, D).transpose(0, 2, 1, 3)
    ).astype(FP8NP)
    wu = np.asarray(gate_up_w, dtype=np.float32).astype(ml_dtypes.bfloat16)
    bd = np.ascontiguousarray(
        np.asarray(gate_down_b, dtype=np.float32).reshape(D, 1))
    bu = np.ascontiguousarray(np.broadcast_to(
        np.asarray(gate_up_b, dtype=np.float32).reshape(1, K), (128, K)).copy())
    we = np.asarray(expert_w, dtype=np.float32) * (scale[:, None] * WSCALE)

    in_maps = []
    for c in range(NC):
        wa = np.empty((K, HS, 128, 2, WA), dtype=FP8NP)
        wb = np.empty((K, HS, 128, 2, WB), dtype=FP8NP)
        for k in range(K):
            blk = we[:, k * V + c * VSH : k * V + (c + 1) * VSH]
            fr = blk.reshape(HS, 2, 128, VSH).transpose(0, 2, 1, 3)
            wa[k] = fr[:, :, :, :WA].astype(FP8NP)
            wb[k] = fr[:, :, :, WA:].astype(FP8NP)
        in_maps.append({"x": x, "wa": wa, "wb": wb, "wd": wd8, "wu": wu,
                        "bd": bd, "bu": bu})

    res = run_bass_kernel_spmd(nc_k, in_maps, core_ids, trace=trace)

    out = np.empty((T, V), dtype=np.float32)
    for c in range(NC):
        out[:, c * VSH : (c + 1) * VSH] = (
            res.results[c]["o"].astype(np.float32).reshape(T, VSH))
    out = out.reshape(B, S, V)
    if trace:
        return out, (res, res)
    return out
